# revision 55
# baseline (speedup 1.0000x reference)
"""Mixtral GQA attention (B=2, S=2048, H=4096, 32 q heads / 8 kv heads,
interleaved RoPE, causal; sliding window 4096 >= S so it is plain causal)
on 8 Trainium2 NeuronCores.

Sharding: DP=2 over batch x TP=4 over kv-head pairs. Core c = 4*b + t
handles batch b, kv heads {2t, 2t+1}, q heads [8t, 8t+8). Each core
computes qkv projection (transposed layout), RoPE, attention, and its
partial of the wo projection; the host sums the 4 partials per batch.

Device layout notes:
 - Everything is computed transposed ([feature, token]) so the PE
   contraction dim always sits on partitions; no on-device transposes
   are needed except V (32 small PE transposes).
 - RoPE is applied neox-style: the wq/wk columns are permuted on the
   host (even dims then odd dims) which turns GPT-J interleaved rotary
   into contiguous half rotations. q.k dot products are invariant.
 - Matmuls run in float32r (fp32 truncated to ~FP22, full PE rate at
   moving-dim >= 256): ~1.5e-4 relative error.
 - softmax skips the max-subtraction (scores are O(10) here), masks the
   upper triangle with affine_select after exp, and normalizes with a
   single all-ones [128,128] stationary matmul that accumulates the
   per-query key-sums broadcast across all 128 partitions, followed by
   a [128,512] reciprocal and one vector multiply straight out of PSUM.
 - The output projection is computed transposed (out^T = wo^T @ attn^T,
   [H, S] in DRAM) so the natural wo layout is the stationary operand
   and the attention output is the moving operand; the host transposes.
"""

import sys

sys.path.insert(0, "/opt/trn_rl_repo")

import numpy as np

import concourse.bass as bass  # noqa: F401
import concourse.mybir as mybir
import concourse.tile as tile
from concourse import bacc
from concourse.bass_utils import run_bass_kernel_spmd

F32 = mybir.dt.float32
F32R = mybir.dt.float32r
BF16 = mybir.dt.bfloat16
U16 = mybir.dt.uint16

B = 2
S = 2048
H = 4096
NH = 32
NKV = 8
HD = 128
GROUP = NH // NKV
ROPE_BASE = 10000.0
SCALE = HD**-0.5

N_CORES = 8
TP = 4  # kv-head-pair groups
Q_PER_CORE = 8
KV_PER_CORE = 2

NC_BLK = Q_PER_CORE + 2 * KV_PER_CORE  # 12 feature blocks of 128 in stage 1
NSEG = 4  # contraction (H) segments
HB = H // 128 // NSEG  # h-blocks per segment = 8
TCH = 4  # token chunks
TC_W = S // TCH  # 512
SB = S // 128  # 16 key blocks

_compiled = None


def _build():
    nc = bacc.Bacc("TRN2", target_bir_lowering=False, debug=False,
                   num_devices=N_CORES)

    hid_t = nc.declare_dram_parameter("hid_t", [H, S], U16,
                                      isOutput=False)  # bf16 bits
    w12 = nc.declare_dram_parameter("w12", [H, NC_BLK * 128], U16,
                                    isOutput=False)  # bf16 bits
    wo = nc.declare_dram_parameter("wo", [Q_PER_CORE * 128, H], U16,
                                   isOutput=False)  # bf16 bits
    cos2 = nc.declare_dram_parameter("cos2", [128, S], F32, isOutput=False)
    sinpm = nc.declare_dram_parameter("sinpm", [128, S], F32, isOutput=False)
    identd = nc.declare_dram_parameter("identd", [128, 128], F32, isOutput=False)
    onesd = nc.declare_dram_parameter("onesd", [128, 128], U16,
                                      isOutput=False)  # bf16 bits
    ltrid = nc.declare_dram_parameter("ltrid", [128, 128], U16,
                                      isOutput=False)  # bf16 bits
    ralld = nc.declare_dram_parameter("ralld", [128, 4 * TC_W], U16,
                                      isOutput=False)  # bf16 bits
    out = nc.declare_dram_parameter("out", [H, S], U16, isOutput=True)

    with tile.TileContext(nc) as tc:
        with tc.tile_pool(name="consts", bufs=1) as consts, \
             tc.tile_pool(name="acc", bufs=1) as accp:
            ident = consts.tile([128, 128], F32R, name="ident", tag="ident")
            ones = consts.tile([128, 128], BF16, name="ones", tag="ones")
            ltri = consts.tile([128, 128], BF16, name="ltri", tag="ltri")
            rall = consts.tile([128, 4, TC_W], BF16, name="rall", tag="rall")
            cost = consts.tile([128, S], F32, name="cost", tag="cost")
            sint = consts.tile([128, S], F32, name="sint", tag="sint")

            acc = [accp.tile([128, S], F32R, name=f"acc{c}", tag=f"acc{c}")
                   for c in range(NC_BLK)]

            # RoPE on one 512-token chunk at a time so attention can start
            # as soon as the first k/q chunks are rotated. k/q0 chunks run
            # on the vector engine inside stage 1's last segment; the other
            # q heads run on the (otherwise idle) gpsimd engine during
            # attention.
            def rope_chunk(c, t, eng, copy_eng=None):
                # 2-input engine ops need equal base partitions for both
                # SBUF inputs, so the half-swap must go through copies
                # (1-input ops may shift partitions).
                lo, hi = t * TC_W, (t + 1) * TC_W
                blk = acc[c][:, lo:hi]
                copy_eng = copy_eng or eng
                pfx = "G" if eng is nc.gpsimd else "D"
                tmp = consts.tile([128, TC_W], F32, name=f"rt{c}_{t}",
                                  tag=f"ropetmp{pfx}{(c * TCH + t) % 3}")
                copy_eng.tensor_copy(tmp[0:64, :], acc[c][64:128, lo:hi])
                copy_eng.tensor_copy(tmp[64:128, :], acc[c][0:64, lo:hi])
                eng.tensor_mul(tmp[:], tmp[:], sint[:, lo:hi])
                eng.tensor_mul(blk, blk, cost[:, lo:hi])
                eng.tensor_add(blk, blk, tmp[:])

            def load_consts():
                nc.sync.dma_start(out=ident[:], in_=identd[:].bitcast(F32R))
                nc.sync.dma_start(out=ones[:], in_=onesd[:].bitcast(BF16))
                nc.sync.dma_start(out=ltri[:], in_=ltrid[:].bitcast(BF16))
                nc.sync.dma_start(
                    out=rall[:],
                    in_=ralld[:].rearrange("p (j f) -> p j f", j=4)
                    .bitcast(BF16))
                nc.sync.dma_start(out=cost[:], in_=cos2[:])
                nc.sync.dma_start(out=sint[:], in_=sinpm[:])

            # ---- stage 1: qkv^T = w12^T @ hid_t over 4 H-segments
            with tc.tile_pool(name="wseg", bufs=NC_BLK + 4) as wp, \
                 tc.tile_pool(name="hidt", bufs=24) as hp, \
                 tc.tile_pool(name="ps1", bufs=8, space="PSUM") as ps1:
                for seg in range(NSEG):
                    wt = [None] * NC_BLK

                    def load_w(c, seg=seg, split=False):
                        w_tile = wp.tile([128, HB, 128], BF16,
                                         name=f"w_{seg}_{c}", tag="w")
                        src = w12[seg * HB * 128:(seg + 1) * HB * 128,
                                  c * 128:(c + 1) * 128] \
                            .rearrange("(hb p) c -> p hb c", p=128) \
                            .bitcast(BF16)
                        if split:
                            # startup: two halves land on two DMA queues so
                            # the first matmul starts sooner.
                            nc.sync.dma_start(out=w_tile[:, 0:HB // 2, :],
                                              in_=src[:, 0:HB // 2, :])
                            nc.sync.dma_start(out=w_tile[:, HB // 2:HB, :],
                                              in_=src[:, HB // 2:HB, :])
                        else:
                            nc.sync.dma_start(out=w_tile[:], in_=src)
                        wt[c] = w_tile

                    # first weight tile before the h tiles so the first
                    # matmul's inputs land ASAP; the rest stream behind.
                    if seg == 0:
                        load_w(0, split=True)
                    for t in range(TCH):
                        ht = []
                        for hb in range(HB):
                            h_tile = hp.tile([128, TC_W], BF16,
                                             name=f"h_{seg}_{t}_{hb}", tag="h")
                            src = hid_t[(seg * HB + hb) * 128:
                                        (seg * HB + hb + 1) * 128,
                                        t * TC_W:(t + 1) * TC_W].bitcast(BF16)
                            if seg == 0 and t == 0:
                                half = TC_W // 2
                                nc.sync.dma_start(out=h_tile[:, 0:half],
                                                  in_=src[:, 0:half])
                                nc.sync.dma_start(out=h_tile[:, half:TC_W],
                                                  in_=src[:, half:TC_W])
                            else:
                                nc.sync.dma_start(out=h_tile[:], in_=src)
                            ht.append(h_tile)
                        if t == 0:
                            for c in range(0 if seg else 1, NC_BLK):
                                load_w(c)
                        if seg == 0 and t == 1:
                            load_consts()
                        for c in range(NC_BLK):
                            pt = ps1.tile([128, TC_W], F32,
                                          name=f"p1_{seg}_{t}_{c}", tag="ps1")
                            for hb in range(HB):
                                nc.tensor.matmul(pt[:], wt[c][:, hb, :], ht[hb][:],
                                                 start=(hb == 0),
                                                 stop=(hb == HB - 1))
                            dst = acc[c][:, t * TC_W:(t + 1) * TC_W]
                            if seg == 0:
                                nc.vector.tensor_copy(dst, pt[:])
                            else:
                                nc.vector.tensor_add(dst, dst, pt[:])
                        if seg == NSEG - 1:
                            # k/q0 RoPE runs in stage 1's tail on the vector
                            # engine, after this chunk's adds so PSUM banks
                            # free first. The other q heads rope during
                            # attention (copies on DVE, muls on gpsimd).
                            for c in (0, Q_PER_CORE, Q_PER_CORE + 1):
                                rope_chunk(c, t, nc.vector)

            # ---- RoPE helper (in place); k blocks now, q blocks pipelined
            with tc.tile_pool(name="vnat", bufs=1) as vp, \
                 tc.tile_pool(name="wop", bufs=3) as wops, \
                 tc.tile_pool(name="outp", bufs=4) as op:
                # ---- stage 2: V natural layout via PE transposes
                vnat = [None] * (KV_PER_CORE * SB)
                with tc.tile_pool(name="ps2", bufs=2, space="PSUM") as ps2:
                    for kv in range(KV_PER_CORE):
                        vt = acc[Q_PER_CORE + KV_PER_CORE + kv]
                        for sb in range(SB):
                            ptt = ps2.tile([128, 128], F32R,
                                           name=f"pt2_{kv}_{sb}", tag="ps2")
                            nc.tensor.transpose(
                                ptt[:],
                                vt[:, sb * 128:(sb + 1) * 128],
                                ident[:],
                            )
                            vtile = vp.tile([128, 128], BF16,
                                            name=f"v{kv}_{sb}", tag=f"v{kv}_{sb}")
                            nc.vector.tensor_copy(vtile[:], ptt[:])
                            vnat[kv * SB + sb] = vtile

                # ---- stage 3: attention per q head; attn overwrites acc[g]
                # ---- stage 4 (interleaved at the end): out^T = wo^T @ attn^T
                with tc.tile_pool(name="probs", bufs=8) as pp, \
                     tc.tile_pool(name="recip", bufs=2) as rcp, \
                     tc.tile_pool(name="prsump", bufs=4) as psp, \
                     tc.tile_pool(name="attnb", bufs=1) as abp, \
                     tc.tile_pool(name="ps_s", bufs=3, space="PSUM") as ps_s, \
                     tc.tile_pool(name="ps_pv", bufs=1, space="PSUM") as ps_pv, \
                     tc.tile_pool(name="ps_sum", bufs=1, space="PSUM") as ps_sm:
                    attn_bf = [abp.tile([128, S], BF16, name=f"ab{g}",
                                        tag=f"ab{g}")
                               for g in range(Q_PER_CORE)]
                    for g in range(Q_PER_CORE):
                        if g + 1 < Q_PER_CORE:
                            # rope head g+1 during head g: copies on DVE,
                            # muls/add on the otherwise-idle gpsimd.
                            for t in range(TCH):
                                rope_chunk(g + 1, t, nc.gpsimd,
                                           copy_eng=nc.vector)
                        kv = g // GROUP
                        kt = acc[Q_PER_CORE + kv]
                        for t in range(TCH):
                            nsb = 4 * t + 4  # key blocks 0 .. 4t+3
                            npr = nsb // 2
                            pv = ps_pv.tile([128, TC_W], F32,
                                            name=f"pv_{g}_{t}", tag="pv")
                            sm = ps_sm.tile([128, TC_W], F32,
                                            name=f"sm_{g}_{t}", tag="sum")
                            qch = acc[g][:, t * TC_W:(t + 1) * TC_W]
                            prs = [None] * npr

                            # diagonal blocks first; the causal mask is a
                            # second matmul accumulating a -1e30 staircase
                            # into the score bank, so exp needs no select.
                            # scores/exp work on PAIRS of key blocks: one
                            # [128,1024] exp per two score banks keeps the
                            # scalar engine off the critical path.
                            def pair_sc(p, g=g, t=t, nsb=nsb, kt=kt, qch=qch,
                                        prs=prs):
                                sc = ps_s.tile([128, 2, TC_W], F32,
                                               name=f"sc_{g}_{t}_{p}", tag="s")
                                for half in range(2):
                                    sb = nsb - 1 - (2 * p + half)
                                    j = sb - 4 * t
                                    nc.tensor.matmul(
                                        sc[:, half, :],
                                        kt[:, sb * 128:(sb + 1) * 128],
                                        qch, start=True, stop=(j < 0))
                                    if j >= 0:
                                        nc.tensor.matmul(sc[:, half, :],
                                                         ltri[:],
                                                         rall[:, j, :],
                                                         start=False, stop=True)
                                pr = pp.tile([128, 2, TC_W], BF16,
                                             name=f"pr_{g}_{t}_{p}", tag="pr")
                                nc.scalar.activation(
                                    pr[:], sc[:],
                                    mybir.ActivationFunctionType.Exp)
                                prs[p] = pr

                            # 2-pair software pipeline: scores run ahead so
                            # exp latency never stalls the PE stream; the
                            # softmax denominator sums pr pairs on the vector
                            # engine, then the all-ones stationary broadcasts
                            # the key-sum into every partition of sm (delayed
                            # one pair so the DVE add is never waited on).
                            pair_sc(0)
                            if npr > 1:
                                pair_sc(1)
                            prsums = []
                            for p in range(npr):
                                if p + 2 < npr:
                                    pair_sc(p + 2)
                                pr = prs[p]
                                for half in range(2):
                                    i = 2 * p + half
                                    sb = nsb - 1 - i
                                    nc.tensor.matmul(
                                        pv[:], vnat[kv * SB + sb][:],
                                        pr[:, half, :], start=(i == 0),
                                        stop=(i == nsb - 1))
                                pst = psp.tile([128, TC_W], BF16,
                                               name=f"psm_{g}_{t}_{p}",
                                               tag="prsum")
                                with nc.allow_low_precision("pr pair sum"):
                                    nc.vector.tensor_add(
                                        pst[:], pr[:, 0, :], pr[:, 1, :])
                                prsums.append(pst)
                                if len(prsums) >= 2:
                                    nc.tensor.matmul(
                                        sm[:], ones[:], prsums[-2][:],
                                        start=(len(prsums) == 2),
                                        stop=False)
                            nc.tensor.matmul(sm[:], ones[:], prsums[-1][:],
                                             start=(len(prsums) == 1),
                                             stop=True)
                            rc = rcp.tile([128, TC_W], F32,
                                          name=f"rc_{g}_{t}", tag="rc")
                            nc.vector.reciprocal_approx_fast(rc[:], sm[:])
                            dst = attn_bf[g][:, t * TC_W:(t + 1) * TC_W]
                            with nc.allow_low_precision("attn out bf16"):
                                nc.vector.tensor_mul(dst, pv[:], rc[:])

                    # ---- stage 4: out^T[n, tok] = sum_g wo_g^T @ attn_g^T
                    for nb in range(H // 128):
                        wn = wops.tile([128, Q_PER_CORE, 128], BF16,
                                       name=f"wo_{nb}", tag="wo")
                        nc.sync.dma_start(
                            out=wn[:],
                            in_=wo[:, nb * 128:(nb + 1) * 128]
                            .rearrange("(g p) c -> p g c", p=128)
                            .bitcast(BF16),
                        )
                        for t in range(TCH):
                            po = ps_s.tile([128, TC_W], F32,
                                           name=f"po_{nb}_{t}", tag="s")
                            for g in range(Q_PER_CORE):
                                nc.tensor.matmul(
                                    po[:],
                                    wn[:, g, :],
                                    attn_bf[g][:, t * TC_W:(t + 1) * TC_W],
                                    start=(g == 0), stop=(g == Q_PER_CORE - 1),
                                )
                            ot = op.tile([128, TC_W], BF16,
                                         name=f"ot_{nb}_{t}", tag="ot")
                            with nc.allow_low_precision("out bf16"):
                                nc.vector.tensor_copy(ot[:], po[:])
                            nc.sync.dma_start(
                                out=out[nb * 128:(nb + 1) * 128,
                                        t * TC_W:(t + 1) * TC_W].bitcast(BF16),
                                in_=ot[:],
                            )

    nc.compile()
    return nc


def _get_compiled():
    global _compiled
    if _compiled is None:
        _compiled = _build()
    return _compiled


_EVEN_ODD = np.concatenate([np.arange(0, HD, 2), np.arange(1, HD, 2)])


def _to_bf16_u16(a):
    """fp32 -> bf16 bit pattern (round to nearest even), as uint16."""
    u = np.ascontiguousarray(a, dtype=np.float32).view(np.uint32)
    rounded = u + 0x7FFF + ((u >> 16) & 1)
    return (rounded >> 16).astype(np.uint16)


def _from_bf16_u16(u):
    return (u.astype(np.uint32) << 16).view(np.float32)


def _prep_core_inputs(hidden_states, positions, wqkv, wo):
    """Returns list of 8 in_maps (core c = 4*b + t)."""
    inv_freq = ROPE_BASE ** (-np.arange(0, HD, 2, dtype=np.float32) / HD)
    ident = np.eye(128, dtype=np.float32)
    ones = np.ones((128, 128), dtype=np.float32)
    # ltri[c, p] = 1 iff c <= p; rall[c, j*TC_W + f] = -1e30 iff f < c + 128j.
    # ltri.T @ rall[:, j, :] then equals -1e30 * max(0, p - f + 128j), i.e. a
    # (ramped) -inf exactly where key > query within diagonal block j.
    ltri = np.triu(np.ones((128, 128), dtype=np.float32))
    cc = np.arange(128)[:, None]
    ff = np.arange(TC_W)[None, :]
    rall = np.concatenate(
        [np.where(ff < cc + 128 * j, np.float32(-1e30), np.float32(0.0))
         for j in range(4)], axis=1).astype(np.float32)

    per_batch = []
    for b in range(B):
        hid_t = _to_bf16_u16(hidden_states[b].T)
        ang = positions[b].astype(np.float32)[:, None] * inv_freq[None, :]
        cos = np.cos(ang).T.astype(np.float32)  # [64, S]
        sin = np.sin(ang).T.astype(np.float32)
        cos2 = np.ascontiguousarray(np.concatenate([cos, cos], axis=0))
        sinpm = np.ascontiguousarray(np.concatenate([-sin, sin], axis=0))
        per_batch.append((hid_t, cos2, sinpm))

    in_maps = []
    for c in range(N_CORES):
        b, t = c // TP, c % TP
        hid_t, cos2, sinpm = per_batch[b]
        blocks = []
        for gh in range(Q_PER_CORE):  # q heads, permuted + pre-scaled
            h = Q_PER_CORE * t + gh
            blocks.append(wqkv[:, h * HD:(h + 1) * HD][:, _EVEN_ODD] * SCALE)
        for m in range(KV_PER_CORE):  # k heads, permuted
            h = KV_PER_CORE * t + m
            blocks.append(
                wqkv[:, NH * HD + h * HD: NH * HD + (h + 1) * HD][:, _EVEN_ODD])
        for m in range(KV_PER_CORE):  # v heads, natural
            h = KV_PER_CORE * t + m
            base = (NH + NKV) * HD
            blocks.append(wqkv[:, base + h * HD: base + (h + 1) * HD])
        w12 = _to_bf16_u16(np.concatenate(blocks, axis=1))
        wo_shard = _to_bf16_u16(
            wo[Q_PER_CORE * HD * t: Q_PER_CORE * HD * (t + 1), :])
        in_maps.append({
            "hid_t": hid_t, "w12": w12, "wo": wo_shard,
            "cos2": cos2, "sinpm": sinpm,
            "identd": ident, "onesd": _to_bf16_u16(ones),
            "ltrid": _to_bf16_u16(ltri), "ralld": _to_bf16_u16(rall),
        })
    return in_maps


def kernel(hidden_states, positions, wqkv, wo):
    hidden_states = np.asarray(hidden_states)
    positions = np.asarray(positions)
    wqkv = np.asarray(wqkv)
    wo = np.asarray(wo)
    nc = _get_compiled()
    in_maps = _prep_core_inputs(hidden_states, positions, wqkv, wo)
    res = run_bass_kernel_spmd(nc, in_maps, list(range(N_CORES)))
    full_t = np.zeros((B, H, S), dtype=np.float32)
    for c in range(N_CORES):
        full_t[c // TP] += _from_bf16_u16(res.results[c]["out"])
    return np.ascontiguousarray(full_t.transpose(0, 2, 1))


# revision 58
# speedup vs baseline: 1.0183x; 1.0183x over previous
"""Mixtral GQA attention (B=2, S=2048, H=4096, 32 q heads / 8 kv heads,
interleaved RoPE, causal; sliding window 4096 >= S so it is plain causal)
on 8 Trainium2 NeuronCores.

Sharding: DP=2 over batch x TP=4 over kv-head pairs. Core c = 4*b + t
handles batch b, kv heads {2t, 2t+1}, q heads [8t, 8t+8). Each core
computes qkv projection (transposed layout), RoPE, attention, and its
partial of the wo projection; the host sums the 4 partials per batch.

Device layout notes:
 - Everything is computed transposed ([feature, token]) so the PE
   contraction dim always sits on partitions; no on-device transposes
   are needed except V (32 small PE transposes).
 - RoPE is applied neox-style: the wq/wk columns are permuted on the
   host (even dims then odd dims) which turns GPT-J interleaved rotary
   into contiguous half rotations. q.k dot products are invariant.
 - Matmuls run in float32r (fp32 truncated to ~FP22, full PE rate at
   moving-dim >= 256): ~1.5e-4 relative error.
 - softmax skips the max-subtraction (scores are O(10) here), masks the
   upper triangle with affine_select after exp, and normalizes with a
   single all-ones [128,128] stationary matmul that accumulates the
   per-query key-sums broadcast across all 128 partitions, followed by
   a [128,512] reciprocal and one vector multiply straight out of PSUM.
 - The output projection is computed transposed (out^T = wo^T @ attn^T,
   [H, S] in DRAM) so the natural wo layout is the stationary operand
   and the attention output is the moving operand; the host transposes.
"""

import sys

sys.path.insert(0, "/opt/trn_rl_repo")

import numpy as np

import concourse.bass as bass  # noqa: F401
import concourse.mybir as mybir
import concourse.tile as tile
from concourse import bacc
from concourse.bass_utils import run_bass_kernel_spmd

F32 = mybir.dt.float32
F32R = mybir.dt.float32r
BF16 = mybir.dt.bfloat16
U16 = mybir.dt.uint16

B = 2
S = 2048
H = 4096
NH = 32
NKV = 8
HD = 128
GROUP = NH // NKV
ROPE_BASE = 10000.0
SCALE = HD**-0.5

N_CORES = 8
TP = 4  # kv-head-pair groups
Q_PER_CORE = 8
KV_PER_CORE = 2

NC_BLK = Q_PER_CORE + 2 * KV_PER_CORE  # 12 feature blocks of 128 in stage 1
NSEG = 4  # contraction (H) segments
HB = H // 128 // NSEG  # h-blocks per segment = 8
TCH = 4  # token chunks
TC_W = S // TCH  # 512
SB = S // 128  # 16 key blocks

_compiled = None


def _build():
    nc = bacc.Bacc("TRN2", target_bir_lowering=False, debug=False,
                   num_devices=N_CORES)

    hid_t = nc.declare_dram_parameter("hid_t", [H, S], U16,
                                      isOutput=False)  # bf16 bits
    w12 = nc.declare_dram_parameter("w12", [H, NC_BLK * 128], U16,
                                    isOutput=False)  # bf16 bits
    wo = nc.declare_dram_parameter("wo", [Q_PER_CORE * 128, H], U16,
                                   isOutput=False)  # bf16 bits
    cos2 = nc.declare_dram_parameter("cos2", [128, S], F32, isOutput=False)
    sinpm = nc.declare_dram_parameter("sinpm", [128, S], F32, isOutput=False)
    identd = nc.declare_dram_parameter("identd", [128, 128], F32, isOutput=False)
    onesd = nc.declare_dram_parameter("onesd", [128, 128], U16,
                                      isOutput=False)  # bf16 bits
    ltrid = nc.declare_dram_parameter("ltrid", [128, 128], U16,
                                      isOutput=False)  # bf16 bits
    ralld = nc.declare_dram_parameter("ralld", [128, 4 * TC_W], U16,
                                      isOutput=False)  # bf16 bits
    out = nc.declare_dram_parameter("out", [H, S], U16, isOutput=True)

    with tile.TileContext(nc) as tc:
        with tc.tile_pool(name="consts", bufs=1) as consts, \
             tc.tile_pool(name="acc", bufs=1) as accp:
            ident = consts.tile([128, 128], F32R, name="ident", tag="ident")
            ones = consts.tile([128, 128], BF16, name="ones", tag="ones")
            ltri = consts.tile([128, 128], BF16, name="ltri", tag="ltri")
            rall = consts.tile([128, 4, TC_W], BF16, name="rall", tag="rall")
            cost = consts.tile([128, S], F32, name="cost", tag="cost")
            sint = consts.tile([128, S], F32, name="sint", tag="sint")

            acc = [accp.tile([128, S], F32R, name=f"acc{c}", tag=f"acc{c}")
                   for c in range(NC_BLK)]

            # RoPE on one 512-token chunk at a time so attention can start
            # as soon as the first k/q chunks are rotated. k/q0 chunks run
            # on the vector engine inside stage 1's last segment; the other
            # q heads run on the (otherwise idle) gpsimd engine during
            # attention.
            def rope_chunk(c, t, eng, copy_eng=None):
                # 2-input engine ops need equal base partitions for both
                # SBUF inputs, so the half-swap must go through copies
                # (1-input ops may shift partitions).
                lo, hi = t * TC_W, (t + 1) * TC_W
                blk = acc[c][:, lo:hi]
                copy_eng = copy_eng or eng
                pfx = "G" if eng is nc.gpsimd else "D"
                tmp = consts.tile([128, TC_W], F32, name=f"rt{c}_{t}",
                                  tag=f"ropetmp{pfx}{(c * TCH + t) % 3}")
                if copy_eng is nc.scalar:
                    nc.scalar.copy(tmp[0:64, :], acc[c][64:128, lo:hi])
                    nc.scalar.copy(tmp[64:128, :], acc[c][0:64, lo:hi])
                else:
                    copy_eng.tensor_copy(tmp[0:64, :], acc[c][64:128, lo:hi])
                    copy_eng.tensor_copy(tmp[64:128, :], acc[c][0:64, lo:hi])
                eng.tensor_mul(tmp[:], tmp[:], sint[:, lo:hi])
                eng.tensor_mul(blk, blk, cost[:, lo:hi])
                eng.tensor_add(blk, blk, tmp[:])

            def load_consts():
                nc.sync.dma_start(out=ident[:], in_=identd[:].bitcast(F32R))
                nc.sync.dma_start(out=ones[:], in_=onesd[:].bitcast(BF16))
                nc.sync.dma_start(out=ltri[:], in_=ltrid[:].bitcast(BF16))
                nc.sync.dma_start(
                    out=rall[:],
                    in_=ralld[:].rearrange("p (j f) -> p j f", j=4)
                    .bitcast(BF16))
                nc.sync.dma_start(out=cost[:], in_=cos2[:])
                nc.sync.dma_start(out=sint[:], in_=sinpm[:])

            # ---- stage 1: qkv^T = w12^T @ hid_t over 4 H-segments
            with tc.tile_pool(name="wseg", bufs=NC_BLK + 4) as wp, \
                 tc.tile_pool(name="hidt", bufs=24) as hp, \
                 tc.tile_pool(name="ps1", bufs=8, space="PSUM") as ps1:
                for seg in range(NSEG):
                    wt = [None] * NC_BLK

                    def load_w(c, seg=seg, split=False):
                        w_tile = wp.tile([128, HB, 128], BF16,
                                         name=f"w_{seg}_{c}", tag="w")
                        src = w12[seg * HB * 128:(seg + 1) * HB * 128,
                                  c * 128:(c + 1) * 128] \
                            .rearrange("(hb p) c -> p hb c", p=128) \
                            .bitcast(BF16)
                        if split:
                            # startup: two halves land on two DMA queues so
                            # the first matmul starts sooner.
                            nc.sync.dma_start(out=w_tile[:, 0:HB // 2, :],
                                              in_=src[:, 0:HB // 2, :])
                            nc.sync.dma_start(out=w_tile[:, HB // 2:HB, :],
                                              in_=src[:, HB // 2:HB, :])
                        else:
                            nc.sync.dma_start(out=w_tile[:], in_=src)
                        wt[c] = w_tile

                    # first weight tile before the h tiles so the first
                    # matmul's inputs land ASAP; the rest stream behind.
                    if seg == 0:
                        load_w(0, split=True)
                    for t in range(TCH):
                        ht = []
                        for hb in range(HB):
                            h_tile = hp.tile([128, TC_W], BF16,
                                             name=f"h_{seg}_{t}_{hb}", tag="h")
                            src = hid_t[(seg * HB + hb) * 128:
                                        (seg * HB + hb + 1) * 128,
                                        t * TC_W:(t + 1) * TC_W].bitcast(BF16)
                            if seg == 0 and t == 0:
                                half = TC_W // 2
                                nc.sync.dma_start(out=h_tile[:, 0:half],
                                                  in_=src[:, 0:half])
                                nc.sync.dma_start(out=h_tile[:, half:TC_W],
                                                  in_=src[:, half:TC_W])
                            else:
                                nc.sync.dma_start(out=h_tile[:], in_=src)
                            ht.append(h_tile)
                        if t == 0:
                            for c in range(0 if seg else 1, NC_BLK):
                                load_w(c)
                        if seg == 0 and t == 1:
                            load_consts()
                        for c in range(NC_BLK):
                            pt = ps1.tile([128, TC_W], F32,
                                          name=f"p1_{seg}_{t}_{c}", tag="ps1")
                            for hb in range(HB):
                                nc.tensor.matmul(pt[:], wt[c][:, hb, :], ht[hb][:],
                                                 start=(hb == 0),
                                                 stop=(hb == HB - 1))
                            dst = acc[c][:, t * TC_W:(t + 1) * TC_W]
                            if seg == 0:
                                nc.vector.tensor_copy(dst, pt[:])
                            else:
                                nc.vector.tensor_add(dst, dst, pt[:])
                        if seg == NSEG - 1:
                            # k/q0 RoPE runs in stage 1's tail, after this
                            # chunk's adds so PSUM banks free first: copies
                            # on the idle scalar engine, muls on DVE. The
                            # other q heads rope during attention.
                            for c in (0, Q_PER_CORE, Q_PER_CORE + 1):
                                rope_chunk(c, t, nc.vector,
                                           copy_eng=nc.scalar)

            # ---- RoPE helper (in place); k blocks now, q blocks pipelined
            with tc.tile_pool(name="vnat", bufs=1) as vp, \
                 tc.tile_pool(name="wop", bufs=3) as wops, \
                 tc.tile_pool(name="outp", bufs=4) as op:
                # ---- stage 2: V natural layout via PE transposes
                vnat = [None] * (KV_PER_CORE * SB)
                with tc.tile_pool(name="ps2", bufs=2, space="PSUM") as ps2:
                    for kv in range(KV_PER_CORE):
                        vt = acc[Q_PER_CORE + KV_PER_CORE + kv]
                        for sb in range(SB):
                            ptt = ps2.tile([128, 128], F32R,
                                           name=f"pt2_{kv}_{sb}", tag="ps2")
                            nc.tensor.transpose(
                                ptt[:],
                                vt[:, sb * 128:(sb + 1) * 128],
                                ident[:],
                            )
                            vtile = vp.tile([128, 128], BF16,
                                            name=f"v{kv}_{sb}", tag=f"v{kv}_{sb}")
                            nc.vector.tensor_copy(vtile[:], ptt[:])
                            vnat[kv * SB + sb] = vtile

                # ---- stage 3: attention per q head; attn overwrites acc[g]
                # ---- stage 4 (interleaved at the end): out^T = wo^T @ attn^T
                with tc.tile_pool(name="probs", bufs=8) as pp, \
                     tc.tile_pool(name="recip", bufs=2) as rcp, \
                     tc.tile_pool(name="prsump", bufs=4) as psp, \
                     tc.tile_pool(name="attnb", bufs=1) as abp, \
                     tc.tile_pool(name="ps_s", bufs=3, space="PSUM") as ps_s, \
                     tc.tile_pool(name="ps_pv", bufs=1, space="PSUM") as ps_pv, \
                     tc.tile_pool(name="ps_sum", bufs=1, space="PSUM") as ps_sm:
                    attn_bf = [abp.tile([128, S], BF16, name=f"ab{g}",
                                        tag=f"ab{g}")
                               for g in range(Q_PER_CORE)]
                    for g in range(Q_PER_CORE):
                        kv = g // GROUP
                        kt = acc[Q_PER_CORE + kv]
                        for t in range(TCH):
                            if g + 1 < Q_PER_CORE:
                                # rope one chunk of head g+1 per chunk of
                                # head g: copies on ACT, muls/add on the
                                # otherwise-idle gpsimd.
                                rope_chunk(g + 1, t, nc.gpsimd,
                                           copy_eng=nc.scalar)
                            nsb = 4 * t + 4  # key blocks 0 .. 4t+3
                            npr = nsb // 2
                            pv = ps_pv.tile([128, TC_W], F32,
                                            name=f"pv_{g}_{t}", tag="pv")
                            sm = ps_sm.tile([128, TC_W], F32,
                                            name=f"sm_{g}_{t}", tag="sum")
                            qch = acc[g][:, t * TC_W:(t + 1) * TC_W]
                            prs = [None] * npr

                            # diagonal blocks first; the causal mask is a
                            # second matmul accumulating a -1e30 staircase
                            # into the score bank, so exp needs no select.
                            # scores/exp work on PAIRS of key blocks: one
                            # [128,1024] exp per two score banks keeps the
                            # scalar engine off the critical path.
                            def pair_sc(p, g=g, t=t, nsb=nsb, kt=kt, qch=qch,
                                        prs=prs):
                                sc = ps_s.tile([128, 2, TC_W], F32,
                                               name=f"sc_{g}_{t}_{p}", tag="s")
                                for half in range(2):
                                    sb = nsb - 1 - (2 * p + half)
                                    j = sb - 4 * t
                                    nc.tensor.matmul(
                                        sc[:, half, :],
                                        kt[:, sb * 128:(sb + 1) * 128],
                                        qch, start=True, stop=(j < 0))
                                    if j >= 0:
                                        nc.tensor.matmul(sc[:, half, :],
                                                         ltri[:],
                                                         rall[:, j, :],
                                                         start=False, stop=True)
                                pr = pp.tile([128, 2, TC_W], BF16,
                                             name=f"pr_{g}_{t}_{p}", tag="pr")
                                nc.scalar.activation(
                                    pr[:], sc[:],
                                    mybir.ActivationFunctionType.Exp)
                                prs[p] = pr

                            # 2-pair software pipeline: scores run ahead so
                            # exp latency never stalls the PE stream; the
                            # softmax denominator sums pr pairs on the vector
                            # engine, then the all-ones stationary broadcasts
                            # the key-sum into every partition of sm (delayed
                            # one pair so the DVE add is never waited on).
                            pair_sc(0)
                            if npr > 1:
                                pair_sc(1)
                            prsums = []
                            for p in range(npr):
                                if p + 2 < npr:
                                    pair_sc(p + 2)
                                pr = prs[p]
                                for half in range(2):
                                    i = 2 * p + half
                                    sb = nsb - 1 - i
                                    nc.tensor.matmul(
                                        pv[:], vnat[kv * SB + sb][:],
                                        pr[:, half, :], start=(i == 0),
                                        stop=(i == nsb - 1))
                                pst = psp.tile([128, TC_W], BF16,
                                               name=f"psm_{g}_{t}_{p}",
                                               tag="prsum")
                                with nc.allow_low_precision("pr pair sum"):
                                    nc.vector.tensor_add(
                                        pst[:], pr[:, 0, :], pr[:, 1, :])
                                prsums.append(pst)
                                if len(prsums) >= 2:
                                    nc.tensor.matmul(
                                        sm[:], ones[:], prsums[-2][:],
                                        start=(len(prsums) == 2),
                                        stop=False)
                            nc.tensor.matmul(sm[:], ones[:], prsums[-1][:],
                                             start=(len(prsums) == 1),
                                             stop=True)
                            rc = rcp.tile([128, TC_W], F32,
                                          name=f"rc_{g}_{t}", tag="rc")
                            nc.vector.reciprocal_approx_fast(rc[:], sm[:])
                            dst = attn_bf[g][:, t * TC_W:(t + 1) * TC_W]
                            with nc.allow_low_precision("attn out bf16"):
                                nc.vector.tensor_mul(dst, pv[:], rc[:])

                    # ---- stage 4: out^T[n, tok] = sum_g wo_g^T @ attn_g^T
                    for nb in range(H // 128):
                        wn = wops.tile([128, Q_PER_CORE, 128], BF16,
                                       name=f"wo_{nb}", tag="wo")
                        nc.sync.dma_start(
                            out=wn[:],
                            in_=wo[:, nb * 128:(nb + 1) * 128]
                            .rearrange("(g p) c -> p g c", p=128)
                            .bitcast(BF16),
                        )
                        for t in range(TCH):
                            po = ps_s.tile([128, TC_W], F32,
                                           name=f"po_{nb}_{t}", tag="s")
                            for g in range(Q_PER_CORE):
                                nc.tensor.matmul(
                                    po[:],
                                    wn[:, g, :],
                                    attn_bf[g][:, t * TC_W:(t + 1) * TC_W],
                                    start=(g == 0), stop=(g == Q_PER_CORE - 1),
                                )
                            ot = op.tile([128, TC_W], BF16,
                                         name=f"ot_{nb}_{t}", tag="ot")
                            with nc.allow_low_precision("out bf16"):
                                nc.vector.tensor_copy(ot[:], po[:])
                            nc.sync.dma_start(
                                out=out[nb * 128:(nb + 1) * 128,
                                        t * TC_W:(t + 1) * TC_W].bitcast(BF16),
                                in_=ot[:],
                            )

    nc.compile()
    return nc


def _get_compiled():
    global _compiled
    if _compiled is None:
        _compiled = _build()
    return _compiled


_EVEN_ODD = np.concatenate([np.arange(0, HD, 2), np.arange(1, HD, 2)])


def _to_bf16_u16(a):
    """fp32 -> bf16 bit pattern (round to nearest even), as uint16."""
    u = np.ascontiguousarray(a, dtype=np.float32).view(np.uint32)
    rounded = u + 0x7FFF + ((u >> 16) & 1)
    return (rounded >> 16).astype(np.uint16)


def _from_bf16_u16(u):
    return (u.astype(np.uint32) << 16).view(np.float32)


def _prep_core_inputs(hidden_states, positions, wqkv, wo):
    """Returns list of 8 in_maps (core c = 4*b + t)."""
    inv_freq = ROPE_BASE ** (-np.arange(0, HD, 2, dtype=np.float32) / HD)
    ident = np.eye(128, dtype=np.float32)
    ones = np.ones((128, 128), dtype=np.float32)
    # ltri[c, p] = 1 iff c <= p; rall[c, j*TC_W + f] = -1e30 iff f < c + 128j.
    # ltri.T @ rall[:, j, :] then equals -1e30 * max(0, p - f + 128j), i.e. a
    # (ramped) -inf exactly where key > query within diagonal block j.
    ltri = np.triu(np.ones((128, 128), dtype=np.float32))
    cc = np.arange(128)[:, None]
    ff = np.arange(TC_W)[None, :]
    rall = np.concatenate(
        [np.where(ff < cc + 128 * j, np.float32(-1e30), np.float32(0.0))
         for j in range(4)], axis=1).astype(np.float32)

    per_batch = []
    for b in range(B):
        hid_t = _to_bf16_u16(hidden_states[b].T)
        ang = positions[b].astype(np.float32)[:, None] * inv_freq[None, :]
        cos = np.cos(ang).T.astype(np.float32)  # [64, S]
        sin = np.sin(ang).T.astype(np.float32)
        cos2 = np.ascontiguousarray(np.concatenate([cos, cos], axis=0))
        sinpm = np.ascontiguousarray(np.concatenate([-sin, sin], axis=0))
        per_batch.append((hid_t, cos2, sinpm))

    in_maps = []
    for c in range(N_CORES):
        b, t = c // TP, c % TP
        hid_t, cos2, sinpm = per_batch[b]
        blocks = []
        for gh in range(Q_PER_CORE):  # q heads, permuted + pre-scaled
            h = Q_PER_CORE * t + gh
            blocks.append(wqkv[:, h * HD:(h + 1) * HD][:, _EVEN_ODD] * SCALE)
        for m in range(KV_PER_CORE):  # k heads, permuted
            h = KV_PER_CORE * t + m
            blocks.append(
                wqkv[:, NH * HD + h * HD: NH * HD + (h + 1) * HD][:, _EVEN_ODD])
        for m in range(KV_PER_CORE):  # v heads, natural
            h = KV_PER_CORE * t + m
            base = (NH + NKV) * HD
            blocks.append(wqkv[:, base + h * HD: base + (h + 1) * HD])
        w12 = _to_bf16_u16(np.concatenate(blocks, axis=1))
        wo_shard = _to_bf16_u16(
            wo[Q_PER_CORE * HD * t: Q_PER_CORE * HD * (t + 1), :])
        in_maps.append({
            "hid_t": hid_t, "w12": w12, "wo": wo_shard,
            "cos2": cos2, "sinpm": sinpm,
            "identd": ident, "onesd": _to_bf16_u16(ones),
            "ltrid": _to_bf16_u16(ltri), "ralld": _to_bf16_u16(rall),
        })
    return in_maps


def kernel(hidden_states, positions, wqkv, wo):
    hidden_states = np.asarray(hidden_states)
    positions = np.asarray(positions)
    wqkv = np.asarray(wqkv)
    wo = np.asarray(wo)
    nc = _get_compiled()
    in_maps = _prep_core_inputs(hidden_states, positions, wqkv, wo)
    res = run_bass_kernel_spmd(nc, in_maps, list(range(N_CORES)))
    full_t = np.zeros((B, H, S), dtype=np.float32)
    for c in range(N_CORES):
        full_t[c // TP] += _from_bf16_u16(res.results[c]["out"])
    return np.ascontiguousarray(full_t.transpose(0, 2, 1))


# revision 60
# speedup vs baseline: 1.0212x; 1.0029x over previous
"""Mixtral GQA attention (B=2, S=2048, H=4096, 32 q heads / 8 kv heads,
interleaved RoPE, causal; sliding window 4096 >= S so it is plain causal)
on 8 Trainium2 NeuronCores.

Sharding: DP=2 over batch x TP=4 over kv-head pairs. Core c = 4*b + t
handles batch b, kv heads {2t, 2t+1}, q heads [8t, 8t+8). Each core
computes qkv projection (transposed layout), RoPE, attention, and its
partial of the wo projection; the host sums the 4 partials per batch.

Device layout notes:
 - Everything is computed transposed ([feature, token]) so the PE
   contraction dim always sits on partitions; no on-device transposes
   are needed except V (32 small PE transposes).
 - RoPE is applied neox-style: the wq/wk columns are permuted on the
   host (even dims then odd dims) which turns GPT-J interleaved rotary
   into contiguous half rotations. q.k dot products are invariant.
 - Matmuls run in float32r (fp32 truncated to ~FP22, full PE rate at
   moving-dim >= 256): ~1.5e-4 relative error.
 - softmax skips the max-subtraction (scores are O(10) here), masks the
   upper triangle with affine_select after exp, and normalizes with a
   single all-ones [128,128] stationary matmul that accumulates the
   per-query key-sums broadcast across all 128 partitions, followed by
   a [128,512] reciprocal and one vector multiply straight out of PSUM.
 - The output projection is computed transposed (out^T = wo^T @ attn^T,
   [H, S] in DRAM) so the natural wo layout is the stationary operand
   and the attention output is the moving operand; the host transposes.
"""

import sys

sys.path.insert(0, "/opt/trn_rl_repo")

import numpy as np

import concourse.bass as bass  # noqa: F401
import concourse.mybir as mybir
import concourse.tile as tile
from concourse import bacc
from concourse.bass_utils import run_bass_kernel_spmd

F32 = mybir.dt.float32
F32R = mybir.dt.float32r
BF16 = mybir.dt.bfloat16
U16 = mybir.dt.uint16

B = 2
S = 2048
H = 4096
NH = 32
NKV = 8
HD = 128
GROUP = NH // NKV
ROPE_BASE = 10000.0
SCALE = HD**-0.5

N_CORES = 8
TP = 4  # kv-head-pair groups
Q_PER_CORE = 8
KV_PER_CORE = 2

NC_BLK = Q_PER_CORE + 2 * KV_PER_CORE  # 12 feature blocks of 128 in stage 1
NSEG = 4  # contraction (H) segments
HB = H // 128 // NSEG  # h-blocks per segment = 8
TCH = 4  # token chunks
TC_W = S // TCH  # 512
SB = S // 128  # 16 key blocks

_compiled = None


def _build():
    nc = bacc.Bacc("TRN2", target_bir_lowering=False, debug=False,
                   num_devices=N_CORES)

    hid_t = nc.declare_dram_parameter("hid_t", [H, S], U16,
                                      isOutput=False)  # bf16 bits
    w12 = nc.declare_dram_parameter("w12", [H, NC_BLK * 128], U16,
                                    isOutput=False)  # bf16 bits
    wo = nc.declare_dram_parameter("wo", [Q_PER_CORE * 128, H], U16,
                                   isOutput=False)  # bf16 bits
    cos2 = nc.declare_dram_parameter("cos2", [128, S], F32, isOutput=False)
    sinpm = nc.declare_dram_parameter("sinpm", [128, S], F32, isOutput=False)
    identd = nc.declare_dram_parameter("identd", [128, 128], F32, isOutput=False)
    onesd = nc.declare_dram_parameter("onesd", [128, 128], U16,
                                      isOutput=False)  # bf16 bits
    ltrid = nc.declare_dram_parameter("ltrid", [128, 128], U16,
                                      isOutput=False)  # bf16 bits
    ralld = nc.declare_dram_parameter("ralld", [128, 4 * TC_W], U16,
                                      isOutput=False)  # bf16 bits
    out = nc.declare_dram_parameter("out", [H, S], U16, isOutput=True)

    with tile.TileContext(nc) as tc:
        with tc.tile_pool(name="consts", bufs=1) as consts, \
             tc.tile_pool(name="acc", bufs=1) as accp:
            ident = consts.tile([128, 128], F32R, name="ident", tag="ident")
            ones = consts.tile([128, 128], BF16, name="ones", tag="ones")
            ltri = consts.tile([128, 128], BF16, name="ltri", tag="ltri")
            rall = consts.tile([128, 4, TC_W], BF16, name="rall", tag="rall")
            cost = consts.tile([128, S], F32, name="cost", tag="cost")
            sint = consts.tile([128, S], F32, name="sint", tag="sint")

            acc = [accp.tile([128, S], F32R, name=f"acc{c}", tag=f"acc{c}")
                   for c in range(NC_BLK)]

            # RoPE on one 512-token chunk at a time so attention can start
            # as soon as the first k/q chunks are rotated. k/q0 chunks run
            # on the vector engine inside stage 1's last segment; the other
            # q heads run on the (otherwise idle) gpsimd engine during
            # attention.
            rope_slot = {"D": 0, "G": 0}

            def rope_chunk(c, t, eng, copy_eng=None):
                # 2-input engine ops need equal base partitions for both
                # SBUF inputs, so the half-swap must go through copies
                # (1-input ops may shift partitions). The tmp slots rotate
                # by emission order so the cross-engine copy->mul->add
                # chains never block on slot reuse.
                lo, hi = t * TC_W, (t + 1) * TC_W
                blk = acc[c][:, lo:hi]
                copy_eng = copy_eng or eng
                pfx = "G" if eng is nc.gpsimd else "D"
                rope_slot[pfx] += 1
                tmp = consts.tile([128, TC_W], BF16, name=f"rt{c}_{t}",
                                  tag=f"ropetmp{pfx}{rope_slot[pfx] % 6}")
                if copy_eng is nc.scalar:
                    nc.scalar.copy(tmp[0:64, :], acc[c][64:128, lo:hi])
                    nc.scalar.copy(tmp[64:128, :], acc[c][0:64, lo:hi])
                else:
                    copy_eng.tensor_copy(tmp[0:64, :], acc[c][64:128, lo:hi])
                    copy_eng.tensor_copy(tmp[64:128, :], acc[c][0:64, lo:hi])
                eng.tensor_mul(tmp[:], tmp[:], sint[:, lo:hi])
                eng.tensor_mul(blk, blk, cost[:, lo:hi])
                eng.tensor_add(blk, blk, tmp[:])

            def load_consts():
                nc.sync.dma_start(out=ident[:], in_=identd[:].bitcast(F32R))
                nc.sync.dma_start(out=ones[:], in_=onesd[:].bitcast(BF16))
                nc.sync.dma_start(out=ltri[:], in_=ltrid[:].bitcast(BF16))
                nc.sync.dma_start(
                    out=rall[:],
                    in_=ralld[:].rearrange("p (j f) -> p j f", j=4)
                    .bitcast(BF16))
                nc.sync.dma_start(out=cost[:], in_=cos2[:])
                nc.sync.dma_start(out=sint[:], in_=sinpm[:])

            # ---- stage 1: qkv^T = w12^T @ hid_t over 4 H-segments
            with tc.tile_pool(name="wseg", bufs=NC_BLK + 4) as wp, \
                 tc.tile_pool(name="hidt", bufs=24) as hp, \
                 tc.tile_pool(name="ps1", bufs=8, space="PSUM") as ps1:
                for seg in range(NSEG):
                    wt = [None] * NC_BLK

                    def load_w(c, seg=seg, split=False):
                        w_tile = wp.tile([128, HB, 128], BF16,
                                         name=f"w_{seg}_{c}", tag="w")
                        src = w12[seg * HB * 128:(seg + 1) * HB * 128,
                                  c * 128:(c + 1) * 128] \
                            .rearrange("(hb p) c -> p hb c", p=128) \
                            .bitcast(BF16)
                        if split:
                            # startup: two halves land on two DMA queues so
                            # the first matmul starts sooner.
                            nc.sync.dma_start(out=w_tile[:, 0:HB // 2, :],
                                              in_=src[:, 0:HB // 2, :])
                            nc.sync.dma_start(out=w_tile[:, HB // 2:HB, :],
                                              in_=src[:, HB // 2:HB, :])
                        else:
                            nc.sync.dma_start(out=w_tile[:], in_=src)
                        wt[c] = w_tile

                    # first weight tile before the h tiles so the first
                    # matmul's inputs land ASAP; the rest stream behind.
                    if seg == 0:
                        load_w(0, split=True)
                    for t in range(TCH):
                        ht = []
                        for hb in range(HB):
                            h_tile = hp.tile([128, TC_W], BF16,
                                             name=f"h_{seg}_{t}_{hb}", tag="h")
                            src = hid_t[(seg * HB + hb) * 128:
                                        (seg * HB + hb + 1) * 128,
                                        t * TC_W:(t + 1) * TC_W].bitcast(BF16)
                            if seg == 0 and t == 0:
                                half = TC_W // 2
                                nc.sync.dma_start(out=h_tile[:, 0:half],
                                                  in_=src[:, 0:half])
                                nc.sync.dma_start(out=h_tile[:, half:TC_W],
                                                  in_=src[:, half:TC_W])
                            else:
                                nc.sync.dma_start(out=h_tile[:], in_=src)
                            ht.append(h_tile)
                        if t == 0:
                            for c in range(0 if seg else 1, NC_BLK):
                                load_w(c)
                        if seg == 0 and t == 1:
                            load_consts()
                        for c in range(NC_BLK):
                            pt = ps1.tile([128, TC_W], F32,
                                          name=f"p1_{seg}_{t}_{c}", tag="ps1")
                            for hb in range(HB):
                                nc.tensor.matmul(pt[:], wt[c][:, hb, :], ht[hb][:],
                                                 start=(hb == 0),
                                                 stop=(hb == HB - 1))
                            dst = acc[c][:, t * TC_W:(t + 1) * TC_W]
                            if seg == 0:
                                nc.vector.tensor_copy(dst, pt[:])
                            else:
                                nc.vector.tensor_add(dst, dst, pt[:])
                        if seg == NSEG - 1:
                            # k/q0 RoPE runs in stage 1's tail, after this
                            # chunk's adds so PSUM banks free first: copies
                            # on the idle scalar engine, muls on DVE. The
                            # other q heads rope during attention.
                            for c in (0, Q_PER_CORE, Q_PER_CORE + 1):
                                rope_chunk(c, t, nc.vector,
                                           copy_eng=nc.scalar)

            # ---- RoPE helper (in place); k blocks now, q blocks pipelined
            with tc.tile_pool(name="vnat", bufs=1) as vp, \
                 tc.tile_pool(name="wop", bufs=3) as wops, \
                 tc.tile_pool(name="outp", bufs=4) as op:
                # ---- stage 2: V natural layout via PE transposes
                vnat = [None] * (KV_PER_CORE * SB)
                with tc.tile_pool(name="ps2", bufs=2, space="PSUM") as ps2:
                    for kv in range(KV_PER_CORE):
                        vt = acc[Q_PER_CORE + KV_PER_CORE + kv]
                        for sb in range(SB):
                            ptt = ps2.tile([128, 128], F32R,
                                           name=f"pt2_{kv}_{sb}", tag="ps2")
                            nc.tensor.transpose(
                                ptt[:],
                                vt[:, sb * 128:(sb + 1) * 128],
                                ident[:],
                            )
                            vtile = vp.tile([128, 128], BF16,
                                            name=f"v{kv}_{sb}", tag=f"v{kv}_{sb}")
                            nc.vector.tensor_copy(vtile[:], ptt[:])
                            vnat[kv * SB + sb] = vtile

                # ---- stage 3: attention per q head; attn overwrites acc[g]
                # ---- stage 4 (interleaved at the end): out^T = wo^T @ attn^T
                with tc.tile_pool(name="probs", bufs=8) as pp, \
                     tc.tile_pool(name="recip", bufs=2) as rcp, \
                     tc.tile_pool(name="prsump", bufs=4) as psp, \
                     tc.tile_pool(name="attnb", bufs=1) as abp, \
                     tc.tile_pool(name="ps_s", bufs=3, space="PSUM") as ps_s, \
                     tc.tile_pool(name="ps_pv", bufs=1, space="PSUM") as ps_pv, \
                     tc.tile_pool(name="ps_sum", bufs=1, space="PSUM") as ps_sm:
                    attn_bf = [abp.tile([128, S], BF16, name=f"ab{g}",
                                        tag=f"ab{g}")
                               for g in range(Q_PER_CORE)]
                    for g in range(Q_PER_CORE):
                        kv = g // GROUP
                        kt = acc[Q_PER_CORE + kv]
                        for t in range(TCH):
                            if g + 1 < Q_PER_CORE:
                                # rope one chunk of head g+1 per chunk of
                                # head g: copies on ACT, muls/add on the
                                # otherwise-idle gpsimd.
                                rope_chunk(g + 1, t, nc.gpsimd,
                                           copy_eng=nc.scalar)
                            nsb = 4 * t + 4  # key blocks 0 .. 4t+3
                            npr = nsb // 2
                            pv = ps_pv.tile([128, TC_W], F32,
                                            name=f"pv_{g}_{t}", tag="pv")
                            sm = ps_sm.tile([128, TC_W], F32,
                                            name=f"sm_{g}_{t}", tag="sum")
                            qch = acc[g][:, t * TC_W:(t + 1) * TC_W]
                            prs = [None] * npr

                            # diagonal blocks first; the causal mask is a
                            # second matmul accumulating a -1e30 staircase
                            # into the score bank, so exp needs no select.
                            # scores/exp work on PAIRS of key blocks: one
                            # [128,1024] exp per two score banks keeps the
                            # scalar engine off the critical path.
                            def pair_sc(p, g=g, t=t, nsb=nsb, kt=kt, qch=qch,
                                        prs=prs):
                                sc = ps_s.tile([128, 2, TC_W], F32,
                                               name=f"sc_{g}_{t}_{p}", tag="s")
                                for half in range(2):
                                    sb = nsb - 1 - (2 * p + half)
                                    j = sb - 4 * t
                                    nc.tensor.matmul(
                                        sc[:, half, :],
                                        kt[:, sb * 128:(sb + 1) * 128],
                                        qch, start=True, stop=(j < 0))
                                    if j >= 0:
                                        nc.tensor.matmul(sc[:, half, :],
                                                         ltri[:],
                                                         rall[:, j, :],
                                                         start=False, stop=True)
                                pr = pp.tile([128, 2, TC_W], BF16,
                                             name=f"pr_{g}_{t}_{p}", tag="pr")
                                nc.scalar.activation(
                                    pr[:], sc[:],
                                    mybir.ActivationFunctionType.Exp)
                                prs[p] = pr

                            # 2-pair software pipeline: scores run ahead so
                            # exp latency never stalls the PE stream; the
                            # softmax denominator sums pr pairs on the vector
                            # engine, then the all-ones stationary broadcasts
                            # the key-sum into every partition of sm (delayed
                            # one pair so the DVE add is never waited on).
                            pair_sc(0)
                            if npr > 1:
                                pair_sc(1)
                            prsums = []
                            for p in range(npr):
                                if p + 2 < npr:
                                    pair_sc(p + 2)
                                pr = prs[p]
                                for half in range(2):
                                    i = 2 * p + half
                                    sb = nsb - 1 - i
                                    nc.tensor.matmul(
                                        pv[:], vnat[kv * SB + sb][:],
                                        pr[:, half, :], start=(i == 0),
                                        stop=(i == nsb - 1))
                                pst = psp.tile([128, TC_W], BF16,
                                               name=f"psm_{g}_{t}_{p}",
                                               tag="prsum")
                                with nc.allow_low_precision("pr pair sum"):
                                    nc.vector.tensor_add(
                                        pst[:], pr[:, 0, :], pr[:, 1, :])
                                prsums.append(pst)
                                if len(prsums) >= 2:
                                    nc.tensor.matmul(
                                        sm[:], ones[:], prsums[-2][:],
                                        start=(len(prsums) == 2),
                                        stop=False)
                            nc.tensor.matmul(sm[:], ones[:], prsums[-1][:],
                                             start=(len(prsums) == 1),
                                             stop=True)
                            rc = rcp.tile([128, TC_W], F32,
                                          name=f"rc_{g}_{t}", tag="rc")
                            nc.vector.reciprocal_approx_fast(rc[:], sm[:])
                            dst = attn_bf[g][:, t * TC_W:(t + 1) * TC_W]
                            with nc.allow_low_precision("attn out bf16"):
                                nc.vector.tensor_mul(dst, pv[:], rc[:])

                    # ---- stage 4: out^T[n, tok] = sum_g wo_g^T @ attn_g^T
                    for nb in range(H // 128):
                        wn = wops.tile([128, Q_PER_CORE, 128], BF16,
                                       name=f"wo_{nb}", tag="wo")
                        nc.sync.dma_start(
                            out=wn[:],
                            in_=wo[:, nb * 128:(nb + 1) * 128]
                            .rearrange("(g p) c -> p g c", p=128)
                            .bitcast(BF16),
                        )
                        for t in range(TCH):
                            po = ps_s.tile([128, TC_W], F32,
                                           name=f"po_{nb}_{t}", tag="s")
                            for g in range(Q_PER_CORE):
                                nc.tensor.matmul(
                                    po[:],
                                    wn[:, g, :],
                                    attn_bf[g][:, t * TC_W:(t + 1) * TC_W],
                                    start=(g == 0), stop=(g == Q_PER_CORE - 1),
                                )
                            ot = op.tile([128, TC_W], BF16,
                                         name=f"ot_{nb}_{t}", tag="ot")
                            with nc.allow_low_precision("out bf16"):
                                nc.vector.tensor_copy(ot[:], po[:])
                            nc.sync.dma_start(
                                out=out[nb * 128:(nb + 1) * 128,
                                        t * TC_W:(t + 1) * TC_W].bitcast(BF16),
                                in_=ot[:],
                            )

    nc.compile()
    return nc


def _get_compiled():
    global _compiled
    if _compiled is None:
        _compiled = _build()
    return _compiled


_EVEN_ODD = np.concatenate([np.arange(0, HD, 2), np.arange(1, HD, 2)])


def _to_bf16_u16(a):
    """fp32 -> bf16 bit pattern (round to nearest even), as uint16."""
    u = np.ascontiguousarray(a, dtype=np.float32).view(np.uint32)
    rounded = u + 0x7FFF + ((u >> 16) & 1)
    return (rounded >> 16).astype(np.uint16)


def _from_bf16_u16(u):
    return (u.astype(np.uint32) << 16).view(np.float32)


def _prep_core_inputs(hidden_states, positions, wqkv, wo):
    """Returns list of 8 in_maps (core c = 4*b + t)."""
    inv_freq = ROPE_BASE ** (-np.arange(0, HD, 2, dtype=np.float32) / HD)
    ident = np.eye(128, dtype=np.float32)
    ones = np.ones((128, 128), dtype=np.float32)
    # ltri[c, p] = 1 iff c <= p; rall[c, j*TC_W + f] = -1e30 iff f < c + 128j.
    # ltri.T @ rall[:, j, :] then equals -1e30 * max(0, p - f + 128j), i.e. a
    # (ramped) -inf exactly where key > query within diagonal block j.
    ltri = np.triu(np.ones((128, 128), dtype=np.float32))
    cc = np.arange(128)[:, None]
    ff = np.arange(TC_W)[None, :]
    rall = np.concatenate(
        [np.where(ff < cc + 128 * j, np.float32(-1e30), np.float32(0.0))
         for j in range(4)], axis=1).astype(np.float32)

    per_batch = []
    for b in range(B):
        hid_t = _to_bf16_u16(hidden_states[b].T)
        ang = positions[b].astype(np.float32)[:, None] * inv_freq[None, :]
        cos = np.cos(ang).T.astype(np.float32)  # [64, S]
        sin = np.sin(ang).T.astype(np.float32)
        cos2 = np.ascontiguousarray(np.concatenate([cos, cos], axis=0))
        sinpm = np.ascontiguousarray(np.concatenate([-sin, sin], axis=0))
        per_batch.append((hid_t, cos2, sinpm))

    in_maps = []
    for c in range(N_CORES):
        b, t = c // TP, c % TP
        hid_t, cos2, sinpm = per_batch[b]
        blocks = []
        for gh in range(Q_PER_CORE):  # q heads, permuted + pre-scaled
            h = Q_PER_CORE * t + gh
            blocks.append(wqkv[:, h * HD:(h + 1) * HD][:, _EVEN_ODD] * SCALE)
        for m in range(KV_PER_CORE):  # k heads, permuted
            h = KV_PER_CORE * t + m
            blocks.append(
                wqkv[:, NH * HD + h * HD: NH * HD + (h + 1) * HD][:, _EVEN_ODD])
        for m in range(KV_PER_CORE):  # v heads, natural
            h = KV_PER_CORE * t + m
            base = (NH + NKV) * HD
            blocks.append(wqkv[:, base + h * HD: base + (h + 1) * HD])
        w12 = _to_bf16_u16(np.concatenate(blocks, axis=1))
        wo_shard = _to_bf16_u16(
            wo[Q_PER_CORE * HD * t: Q_PER_CORE * HD * (t + 1), :])
        in_maps.append({
            "hid_t": hid_t, "w12": w12, "wo": wo_shard,
            "cos2": cos2, "sinpm": sinpm,
            "identd": ident, "onesd": _to_bf16_u16(ones),
            "ltrid": _to_bf16_u16(ltri), "ralld": _to_bf16_u16(rall),
        })
    return in_maps


def kernel(hidden_states, positions, wqkv, wo):
    hidden_states = np.asarray(hidden_states)
    positions = np.asarray(positions)
    wqkv = np.asarray(wqkv)
    wo = np.asarray(wo)
    nc = _get_compiled()
    in_maps = _prep_core_inputs(hidden_states, positions, wqkv, wo)
    res = run_bass_kernel_spmd(nc, in_maps, list(range(N_CORES)))
    full_t = np.zeros((B, H, S), dtype=np.float32)
    for c in range(N_CORES):
        full_t[c // TP] += _from_bf16_u16(res.results[c]["out"])
    return np.ascontiguousarray(full_t.transpose(0, 2, 1))
